# revision 16
# baseline (speedup 1.0000x reference)
"""Trainium2 Bass kernel for nn_DressedQuantumCircuit.

The 4-qubit dressed quantum circuit in the reference collapses to a
closed form.  With theta_q = (pi/2) * tanh(x_q) and w = q_params:

    out[:, 0] = -sin(w0) * (1/2)     * cos(theta_1 + pi/4)
    out[:, 1] = -sin(w1) * (sqrt2/2) * cos(theta_3 + pi/4)
    out[:, 2] = -sin(w2) * (sqrt2/2) * cos(theta_0)
    out[:, 3] = -sin(w3) * (1/2)     * cos(theta_2 + pi/4)

(derivation: the H + RZ + CRZ layers produce a uniform-magnitude state
with diagonal phases; SWAPs permute wires; RY(w) conjugates Z into
cos(w)Z - sin(w)X, <Z> = 0 and <X_q> reduces to the cosines above.)

So the device kernel is a purely elementwise map over [B, 4] float32:
tanh (ACT), sin with affine prescale (ACT, cos via +pi/2 bias), and a
per-column multiply (DVE).  Pure data parallel over the batch: each of
the 8 cores processes B/8 rows; no cross-device communication.
"""

import math

import numpy as np

import concourse.bass as bass
import concourse.bacc as bacc
import concourse.mybir as mybir
from contextlib import ExitStack
from concourse.bass_utils import run_bass_kernel_spmd
from concourse.tile import TileContext

N_CORES = 8
BATCH = 524288
NQ = 4
B_LOCAL = BATCH // N_CORES          # 65536 rows per core
P = 128                             # SBUF partitions
FREE = B_LOCAL * NQ // P            # 2048 f32 per partition
NCHUNK = 2                          # pipeline chunks per core
C = FREE // NCHUNK                  # 1024 f32 per partition per chunk

# out column j reads input column PERM[j]
PERM = (1, 3, 0, 2)
# cos(t + b) = sin(t + b + pi/2); column 2 has b = 0, others pi/4
SIN_BIAS = (0.75 * math.pi, 0.75 * math.pi, 0.5 * math.pi, 0.75 * math.pi)
# static part of the output coefficients (times -sin(w_j) at runtime)
COEF = (0.5, math.sqrt(2.0) / 2.0, math.sqrt(2.0) / 2.0, 0.5)

TRACE = False          # set by test.py to capture an NTFF profile
LAST_RESULT = None     # BassKernelResults of the last run when TRACE

_cached_nc = None


def _build():
    global _cached_nc
    if _cached_nc is not None:
        return _cached_nc

    nc = bacc.Bacc(trn_type="TRN2")
    x = nc.declare_dram_parameter("x", [B_LOCAL, NQ], mybir.dt.float32, isOutput=False)
    # per-partition constants: cols 0-3 = output coefs A_j
    acoef = nc.declare_dram_parameter(
        "acoef", [P, NQ], mybir.dt.float32, isOutput=False
    )
    y = nc.declare_dram_parameter("y", [B_LOCAL, NQ], mybir.dt.float32, isOutput=True)

    # register the sin biases as const APs (preamble memsets, like Bass.__init__)
    for val in sorted(set(SIN_BIAS)):
        t = nc.alloc_sbuf_tensor(f"const-f32-{val}", [P, 1], mybir.dt.float32)
        nc.gpsimd.memset(t.ap(), val)
        nc.const_aps.aps[(mybir.dt.float32, val)] = t.ap()
    nc.all_engine_barrier()

    # flat views: partition p holds 512 consecutive rows (x4 cols, interleaved)
    xv = x.rearrange("(p n) f -> p (n f)", p=P)   # [128, 2048]
    yv = y.rearrange("(p n) f -> p (n f)", p=P)

    AF = mybir.ActivationFunctionType

    with TileContext(nc) as tc, ExitStack() as ctx:
        apool = ctx.enter_context(tc.tile_pool(name="apool", bufs=1))
        xpool = ctx.enter_context(tc.tile_pool(name="xpool", bufs=2))
        tpool = ctx.enter_context(tc.tile_pool(name="tpool", bufs=2))
        ypool = ctx.enter_context(tc.tile_pool(name="ypool", bufs=2))
        opool = ctx.enter_context(tc.tile_pool(name="opool", bufs=2))

        at = apool.tile([P, NQ], mybir.dt.float32)
        nc.sync.dma_start(at[:], acoef[:])
        # bounce through the scalar engine so each mul needs only ONE wait
        # (the Activation sem covers both this copy and the sins; the
        # TensorScalar encoding has a single sync-wait slot)
        at2 = apool.tile([P, NQ], mybir.dt.float32)
        nc.scalar.copy(at2[:], at[:])

        for i in range(NCHUNK):
            xt = xpool.tile([P, C], mybir.dt.float32)
            nc.sync.dma_start(xt[:], xv[:, bass.ts(i, C)])

            tt = tpool.tile([P, C], mybir.dt.float32)
            nc.scalar.activation(tt[:], xt[:], AF.Tanh)

            yt = ypool.tile([P, C], mybir.dt.float32)
            for j in range(NQ):
                # out col j <- sin((pi/2) * tanh(x_{PERM[j]}) + bias_j)
                nc.scalar.activation(
                    yt[:, j::NQ],
                    tt[:, PERM[j]::NQ],
                    AF.Sin,
                    bias=SIN_BIAS[j],
                    scale=0.5 * math.pi,
                )
            ot = opool.tile([P, C], mybir.dt.float32)
            for j in range(NQ):
                nc.vector.tensor_scalar_mul(
                    ot[:, j::NQ], yt[:, j::NQ], at2[:, j : j + 1]
                )

            nc.sync.dma_start(yv[:, bass.ts(i, C)], ot[:])

    nc.finalize()  # Bacc: runs compile() incl. the 1-wait-per-inst split
    _cached_nc = nc
    return nc


def kernel(input_features: np.ndarray, q_params: np.ndarray) -> np.ndarray:
    global LAST_RESULT
    x = np.ascontiguousarray(np.asarray(input_features, dtype=np.float32))
    w = np.asarray(q_params, dtype=np.float64).reshape(NQ)
    assert x.shape == (BATCH, NQ), x.shape

    # runtime output coefficients, replicated across partitions
    a = -np.sin(w) * np.array(COEF, dtype=np.float64)
    a_rep = np.ascontiguousarray(np.tile(a[None, :], (P, 1)).astype(np.float32))

    nc = _build()
    shards = x.reshape(N_CORES, B_LOCAL, NQ)
    in_maps = [{"x": shards[i], "acoef": a_rep} for i in range(N_CORES)]

    res = run_bass_kernel_spmd(nc, in_maps, list(range(N_CORES)), trace=TRACE)
    if TRACE:
        LAST_RESULT = res

    out = np.concatenate([res.results[i]["y"] for i in range(N_CORES)], axis=0)
    return out.astype(np.float32, copy=False)
